# revision 3
# baseline (speedup 1.0000x reference)
"""Fan-beam FBP on 8 TRN2 cores — row-pair windowed gather with per-unit
variable window width (compile-time class schedule).

kernel3 structure (PE-built tables, DMA broadcast, PE-PSUM accumulate,
row-pair shared window bases) plus: each (view-slot, block, half) unit uses
the narrowest window class DW in {6,10,14,18} that covers its measured
in-window offsets, cutting the DVE field multiply+reduce (the pacing
engine) by ~30%.  Views are assigned to cores STRIDED (core c gets views
8k+c) so one compile-time schedule (max need over the 8 adjacent views of
a slot) is valid for every core of the SPMD module.
"""
import functools
import os
import numpy as np

V, U = 720, 736
NX = NY = 512
SVOX = 400.0
DU = 1.2858
DSO, DOD = 595.0, 490.6
DSD = DSO + DOD
DBETA = 2.0 * np.pi / V

N_CORES = 8
PAIRS = 45
XBLK = 4
G = 8
DWMAX = 18
NELEM = 368
HALF = 2
NW = 512 // G         # 64 windows per row
NWH = NW // HALF      # 32 windows per row per half
NIDXW = 8 * NWH       # 256 gather indices per instruction (8 row-pairs)
UPAD = 768
CLASSES = (6, 10, 14, 18)
CLOFF = {6: 0, 10: NELEM * 6, 14: NELEM * 16, 18: NELEM * 30}
TABW = NELEM * 48     # concatenated class tables width (17664)
MAXW = NWH * G * DWMAX  # 4608
GAW = NWH * DWMAX       # 576

_last_exec_ns = None


def _ramp_h():
    n = np.arange(-(U - 1), U)
    h = np.zeros(2 * U - 1, np.float64)
    h[U - 1] = 1.0 / (4.0 * DU * DU)
    odd = (np.abs(n) % 2 == 1)
    h[odd] = -1.0 / (np.pi * n[odd] * DU) ** 2
    return h


def _geom(v):
    f32 = np.float32
    dx = f32(SVOX / NX)
    xs = (np.arange(NX, dtype=f32) - f32((NX - 1) / 2.0)) * dx
    X = xs[:, None]
    Y = xs[None, :]
    beta = f32(v * DBETA)
    cb = np.cos(beta, dtype=f32)
    sb = np.sin(beta, dtype=f32)
    t = X * cb + Y * sb
    s = -X * sb + Y * cb
    D = f32(DSO) - s
    idxf = f32(DSD) * t / D / f32(DU) + f32((U - 1) / 2.0)
    mask = (idxf >= 0.0) & (idxf <= U - 1.0)
    i0 = np.clip(np.floor(idxf), 0, U - 2)
    f = idxf - i0.astype(f32)
    w = (f32(DSO) / D) ** 2
    aw = (w * (1.0 - f) * mask).astype(np.float16)
    bw = (w * f * mask).astype(np.float16)
    return i0.astype(np.int32), aw, bw


@functools.lru_cache(maxsize=1)
def _schedule():
    """cl[k, blk, hh] = window class for slot k (covers views 8k..8k+7)."""
    need = np.zeros((360, XBLK, HALF), np.int32)
    for v in range(360):
        i0, _, _ = _geom(v)
        q = i0.reshape(NX, NW, G)
        b2 = q.reshape(NX // 2, 2, NW, G).min(axis=(1, 3))
        b2 -= b2 & 1
        lam = q.reshape(NX // 2, 2, NW, G) - b2[:, None, :, None]
        need[v] = lam.max(axis=(1, 3)).reshape(XBLK, 64, HALF, NWH
                                               ).max(axis=(1, 3)) + 2
    sneed = need.reshape(PAIRS, 8, XBLK, HALF).max(axis=1)
    cl = np.full(sneed.shape, DWMAX, np.int32)
    for c in sorted(CLASSES, reverse=True):
        cl[sneed <= c] = c
    return cl


@functools.lru_cache(maxsize=1)
def _host_tables():
    h = _ramp_h()
    k = np.arange(U)
    j = np.arange(U)
    Hm = h[(U - 1) + j[None, :] - k[:, None]]
    us = (k - (U - 1) / 2.0) * DU
    cosw = DSD / np.sqrt(DSD * DSD + us * us)
    Hm = (cosw[:, None] * Hm) * DU * (0.5 * DBETA)
    Hp = np.zeros((U, 2 * NELEM + DWMAX), np.float64)
    Hp[:, :U] = Hm
    # concatenated per-class windowed filter matrices
    H_win = np.zeros((UPAD, TABW), np.float32)
    for dw in CLASSES:
        o = CLOFF[dw]
        for w in range(dw):
            H_win[:U, o + w::dw][:, :NELEM] = \
                Hp[:, w:w + 2 * NELEM:2].astype(np.float32)

    cl = _schedule()
    idx_packed = np.empty((N_CORES, PAIRS, XBLK, HALF, 128, NIDXW // 16),
                          np.int16)
    w_packed = np.zeros((N_CORES, PAIRS, XBLK, HALF, 128, MAXW), np.float16)

    for c in range(N_CORES):
        for a in range(PAIRS):
            v = 8 * a + c
            i0, aw, bw = _geom(v)
            quad = i0.reshape(NX, NW, G)
            base2 = quad.reshape(NX // 2, 2, NW, G).min(axis=(1, 3))
            base2 -= base2 & 1
            base = np.repeat(base2, 2, axis=0)
            lam = (quad - base[:, :, None]).astype(np.int64)

            for blk in range(XBLK):
                for hh in range(HALF):
                    dw = int(cl[a, blk, hh])
                    rs = slice(blk * 128, (blk + 1) * 128)
                    ws = slice(hh * NWH, (hh + 1) * NWH)
                    lamu = lam[rs, ws]                       # [128, NWH, G]
                    Wq = np.zeros((128, NWH, G, dw), np.float16)
                    awu = aw[rs, hh * 256:(hh + 1) * 256].reshape(
                        128, NWH, G, 1)
                    bwu = bw[rs, hh * 256:(hh + 1) * 256].reshape(
                        128, NWH, G, 1)
                    np.put_along_axis(Wq, lamu[..., None], awu, axis=3)
                    np.put_along_axis(Wq, lamu[..., None] + 1, bwu, axis=3)
                    w_packed[c, a, blk, hh, :, :NWH * G * dw] = \
                        Wq.reshape(128, -1)

                    B = (base2[64 * blk:64 * (blk + 1), ws] >> 1
                         ).astype(np.int16)                  # [64 rp, NWH]
                    flat = B.reshape(8, 8 * NWH)             # [g, j]
                    wrapped = flat.reshape(8, NIDXW // 16, 16)
                    idx_packed[c, a, blk, hh] = wrapped.transpose(
                        0, 2, 1).reshape(128, NIDXW // 16)

    return H_win, idx_packed, w_packed


@functools.lru_cache(maxsize=1)
def _build_module():
    import concourse.bacc as bacc
    import concourse.mybir as mybir
    import concourse.tile as tile
    from concourse import library_config

    f32 = mybir.dt.float32
    f16 = mybir.dt.float16
    i16 = mybir.dt.int16
    cl = _schedule()

    nc = bacc.Bacc("TRN2", target_bir_lowering=False, debug=False,
                   num_devices=N_CORES)
    sinoT_d = nc.dram_tensor("sinot", [UPAD, 2 * PAIRS], f32,
                             kind="ExternalInput")
    hwin_d = nc.dram_tensor("hwin", [UPAD, TABW], f32, kind="ExternalInput")
    ident_d = nc.dram_tensor("ident", [128, 128], f32, kind="ExternalInput")
    idx_d = nc.dram_tensor("idxs", [PAIRS, XBLK, HALF, 128, NIDXW // 16],
                           i16, kind="ExternalInput")
    w_d = nc.dram_tensor("wq", [PAIRS, XBLK, HALF, 128, MAXW], f16,
                         kind="ExternalInput")
    qfw_d = nc.dram_tensor("qfwscratch", [2 * PAIRS, 8, TABW], f16,
                           kind="ExternalOutput")
    out_d = nc.dram_tensor("out", [2, NX, NY], f32, kind="ExternalOutput")

    with tile.TileContext(nc) as tc:
        nc.gpsimd.load_library(library_config.ap_gather)
        with (
            tc.tile_pool(name="const", bufs=1) as constp,
            tc.tile_pool(name="hstream", bufs=2) as hstp,
            tc.tile_pool(name="acc", bufs=1, space="PSUM") as accp,
            tc.tile_pool(name="fpsum", bufs=2, space="PSUM") as fpsump,
            tc.tile_pool(name="t2", bufs=3) as t2p,
            tc.tile_pool(name="stream", bufs=3) as strp,
            tc.tile_pool(name="ot", bufs=2) as otp,
            tc.tile_pool(name="g", bufs=4) as gp,
            tc.tile_pool(name="m", bufs=2) as mp,
            tc.tile_pool(name="red", bufs=3) as redp,
            tc.tile_pool(name="fin", bufs=2) as finp,
        ):
            sin_sb = constp.tile([128, 6 * 2 * PAIRS], f32)
            for i in range(6):
                nc.sync.dma_start(
                    sin_sb[:, i * 2 * PAIRS:(i + 1) * 2 * PAIRS],
                    sinoT_d.ap()[i * 128:(i + 1) * 128, :])
            ident = constp.tile([128, 128], f32)
            nc.sync.dma_start(ident[:], ident_d.ap())

            # ---- filter matmul -> concatenated class tables ----
            qf_win = constp.tile([2 * PAIRS, TABW], f16)
            NCHUNK = (TABW + 511) // 512
            for jc in range(NCHUNK):
                n0 = jc * 512
                n1 = min(TABW, n0 + 512)
                hc = hstp.tile([128, 6 * 512], f32, tag="hc")
                for i in range(6):
                    nc.sync.dma_start(
                        hc[:, i * 512:i * 512 + (n1 - n0)],
                        hwin_d.ap()[i * 128:(i + 1) * 128, n0:n1])
                ps = fpsump.tile([2 * PAIRS, 512], f32, tag="filt")
                for kt in range(6):
                    nc.tensor.matmul(
                        ps[:, :n1 - n0],
                        sin_sb[:, kt * 2 * PAIRS:(kt + 1) * 2 * PAIRS],
                        hc[:, kt * 512:kt * 512 + (n1 - n0)],
                        start=(kt == 0), stop=(kt == 5))
                nc.vector.tensor_copy(qf_win[:, n0:n1], ps[:, :n1 - n0])

            for r in range(8):
                nc.sync.dma_start(qfw_d.ap()[:, r, :], qf_win[:])

            # ---- main: 4 block-passes ----
            for b in range(XBLK):
                acc_a = accp.tile([128, 512], f32, tag="acc0")
                acc_b = accp.tile([128, 512], f32, tag="acc1")
                acc_c = accp.tile([128, 512], f32, tag="acc2")
                acc_d = accp.tile([128, 512], f32, tag="acc3")
                acc_t = [acc_a, acc_b, acc_c, acc_d]
                for a in range(PAIRS):
                    cls_used = sorted(set(int(cl[a, b, hh])
                                          for hh in range(HALF)))
                    tins = {}
                    for dw in cls_used:
                        wdt = NELEM * dw
                        T = t2p.tile([128, NELEM * DWMAX], f16, tag="T")
                        for par in (0, 1):
                            dst = T[:].copy()
                            dst.ap = type(dst.ap)(
                                [[16 * NELEM * DWMAX, 8], [1, wdt]])
                            dst.offset = par * NELEM * DWMAX
                            nc.sync.dma_start(
                                dst,
                                qfw_d.ap()[par * PAIRS + a][
                                    :, CLOFF[dw]:CLOFF[dw] + wdt])
                        tins[dw] = T

                    for hh in sorted(range(HALF),
                                     key=lambda h: int(cl[a, b, h])):
                        dw = int(cl[a, b, hh])
                        wlen = NWH * G * dw
                        glen = NWH * dw
                        olen = NIDXW * dw

                        it = strp.tile([128, NIDXW // 16], i16, tag="idx")
                        nc.sync.dma_start(it[:], idx_d.ap()[a, b, hh])
                        wt = strp.tile([128, MAXW], f16, tag="wq")
                        nc.sync.dma_start(wt[:, :wlen],
                                          w_d.ap()[a, b, hh][:, :wlen])

                        ot = otp.tile([128, NIDXW * DWMAX], f16, tag="ot")
                        nc.gpsimd.ap_gather(
                            ot[:, :olen], tins[dw][:, :NELEM * dw], it[:],
                            channels=128, num_elems=NELEM, d=dw,
                            num_idxs=NIDXW)

                        for par in (0, 1):
                            ga = gp.tile([128, GAW], f16, tag=f"g{par}")
                            for lsb in (0, 1):
                                src = ot[:].copy()
                                src.ap = type(src.ap)(
                                    [[16 * NIDXW * DWMAX, 8], [glen, 8],
                                     [1, glen]])
                                src.offset = par * NIDXW * DWMAX
                                dst = ga[:].copy()
                                dst.ap = type(dst.ap)(
                                    [[2 * GAW, 64], [1, glen]])
                                dst.offset = lsb * GAW
                                nc.scalar.dma_start(dst, src)

                            gread = ga[:].copy()
                            gread.ap = type(gread.ap)(
                                [[GAW, 128], [dw, NWH], [0, G], [1, dw]])
                            m = mp.tile([128, MAXW], f16, tag=f"m{par}")
                            nc.vector.tensor_mul(m[:, :wlen], gread,
                                                 wt[:, :wlen])
                            red = redp.tile([128, NWH * G], f32,
                                            tag=f"r{par}")
                            mr = m[:].copy()
                            mr.ap = type(mr.ap)(
                                [[MAXW, 128], [dw, NWH * G], [1, dw]])
                            nc.vector.reduce_sum(red[:], mr,
                                                 axis=mybir.AxisListType.X)
                            nc.tensor.matmul(
                                acc_t[par * 2 + hh][:, 0:256],
                                ident[:], red[:],
                                start=(a == 0), stop=(a == PAIRS - 1))

                for par in (0, 1):
                    s32 = finp.tile([128, 512], f32, tag="fin")
                    nc.vector.tensor_copy(s32[:, 0:256],
                                          acc_t[par * 2][:, 0:256])
                    nc.vector.tensor_copy(s32[:, 256:512],
                                          acc_t[par * 2 + 1][:, 0:256])
                    nc.sync.dma_start(
                        out_d.ap()[par, b * 128:(b + 1) * 128, :], s32[:])

    nc.compile()
    return nc


def kernel(sinogram: np.ndarray) -> np.ndarray:
    global _last_exec_ns
    from concourse import bass_utils

    H_win, idx_packed, w_packed = _host_tables()
    nc = _build_module()

    sino = np.asarray(sinogram, np.float32).reshape(V, U)
    ident = np.eye(128, dtype=np.float32)
    in_maps = []
    for c in range(N_CORES):
        vs = np.concatenate([8 * np.arange(PAIRS) + c,
                             360 + 8 * np.arange(PAIRS) + c])
        st = np.zeros((UPAD, 2 * PAIRS), np.float32)
        st[:U, :] = sino[vs, :].T
        in_maps.append({
            "sinot": st,
            "hwin": H_win,
            "ident": ident,
            "idxs": idx_packed[c],
            "wq": w_packed[c],
        })

    trace = bool(int(os.environ.get("FBP_TRACE", "0")))
    kw = {}
    if trace:
        import tempfile
        kw = dict(trace=True, tmpdir=tempfile.mkdtemp())
    res = bass_utils.run_bass_kernel_spmd(nc, in_maps,
                                          core_ids=list(range(N_CORES)), **kw)
    _last_exec_ns = res.exec_time_ns

    img = np.zeros((NX, NY), np.float64)
    for c in range(N_CORES):
        o = res.results[c]["out"]
        img += o[0]
        img += o[1][::-1, ::-1]
    return img.astype(np.float32).reshape(1, 1, NX, NY)
